# revision 60
# baseline (speedup 1.0000x reference)
"""Trainium2 Bass kernel for nn_GraphVToS_9388798509586 (gnn_message_passing).

Math (per batch element b):
    out[i,j,k] = relu( sum_c d[i,j,c] * (p[i,c,k] + q[j,c,k]) )
    p = vf @ w_vs[:F]
    q = vf @ w_vs[F:] + b_vs     (bias folded: sum_c d[i,j,c]*b[k])

Sharding: data-parallel over batch B=8, one element per NeuronCore.

Per-core device schedule (dense [j, (i,k)] accumulation layout):
  - PE computes projections p, q (6 small matmuls, bias via ones-row).
  - Pairwise term B (sum_c d[i,j,c]*q[j,c,k], elementwise in j) is computed
    as three broadcast products t_c[j,(i,k)] = d[i,j,c]*q[j,c,k] on
    DVE/GPSIMD. d is host-duplicated x2 along k-pairs so BOTH operands read
    innermost step-1 bf16, which unlocks the DVE 2x perf mode. The c-sum
    happens FOR FREE in PSUM by streaming each t_c through the PE with an
    identity stationary (out += I.T @ t_c).
  - Term A (sum_c d[i,j,c]*p[i,c,k]) is ONE matmul per 8-i PSUM bank:
    stationary dT8 [24,128] (rows 3*il+c = d[i0+il,:,c]) against a
    block-diagonal moving operand p_blk [24, (il,k)] holding p[i0+il,c,k]
    in slot il and zeros elsewhere. N=512 streams hide all LDWEIGHTS.
  - ACT drains PSUM with fused ReLU to bf16; 4 batched DMAs out.
  - Software pipeline: term-A + products for chunk g overlap; identity-sums
    for g-1 follow.
Output leaves the device as out[j, i, k] bf16; the host transposes to
[i,j,k] and casts to f32 (layout/gather only, no math).

kernel() is self-contained: hardcoded shapes, host-side shard prep + gather.
"""

import os

import numpy as np

B, N, C, F, K = 8, 128, 3, 64, 64
_N_CORES = 8

NB = 8  # i's per PSUM bank (8*64 = 512 cols)
SC = 16  # i's per super-chunk (2 banks = 1 double-bank PSUM tile)

# blob1 (small, loads first; feeds projections): [vfT | wp | wq]
_VFT_OFF = 0  # rows 0:65: vfT[f, c*128+n]   cols 0:384
_WP_OFF = 384  # rows 0:64                     cols 384:448
_WQ_OFF = 448  # rows 0:65                     cols 448:512
_BLOB1_W = 512
# blob2: [d_dup2 | ident]
_DD_OFF = 0  # [j, (c,i,k2)] = d[i,j,c] duplicated x2   cols 0:768
_ID_OFF = 768  # identity 128x128                         cols 768:896
_BLOB2_W = 896

_BASS_READY = None
_CACHE = {}


def _import_bass():
    global _BASS_READY
    if _BASS_READY is None:
        try:
            import sys

            for p in ("/opt/trn_rl_repo",):
                if p not in sys.path:
                    sys.path.insert(0, p)
            import concourse.bass  # noqa: F401

            _BASS_READY = True
        except Exception:
            _BASS_READY = False
    return _BASS_READY


def _maybe_patch_ldw_opt():
    """Flip walrus --enable-ldw-opt. Kill switch: KERNEL_LDW_OPT=0."""
    import concourse.bass_utils as _bu

    if getattr(_bu, "_ldwopt_patched", False):
        return
    if os.environ.get("KERNEL_LDW_OPT", "1") != "1":
        return
    _orig = _bu.get_walrus_args

    def _gwa(*a, **k):
        return [
            x.replace("--enable-ldw-opt=false", "--enable-ldw-opt=true")
            for x in _orig(*a, **k)
        ]

    _bu.get_walrus_args = _gwa
    _bu._ldwopt_patched = True


def _legalize_waits(nc):
    """Split multi-semaphore waits onto same-engine NOP carriers.

    This walrus build encodes at most ONE sync-wait per compute instruction
    (setupSyncWait raises "Too many sync wait commands" otherwise), and the
    Tile scheduler happily emits 2-3. Inserting a NOP right before the
    instruction on the same engine is semantics-preserving: the engine would
    have blocked at that point anyway.
    """
    import concourse.mybir as mybir

    nop_ctr = [0]

    def make_nop(engine):
        bi = nc.engines[engine].nop(nofuse=True)
        inst = bi.ins
        for f in nc.m.functions:
            for blk in f.blocks:
                try:
                    blk.instructions.remove(inst)
                except ValueError:
                    pass
        inst.name = f"{inst.name}-legalize-{nop_ctr[0]}"
        nop_ctr[0] += 1
        return inst

    for f in nc.m.functions:
        for blk in f.blocks:
            insts = blk.instructions
            idx = 0
            while idx < len(insts):
                inst = insts[idx]
                si = inst.sync_info
                waits = list(si.on_wait) if si is not None and si.on_wait else []
                if len(waits) > 1:
                    for w in waits[:-1]:
                        nop = make_nop(inst.engine)
                        nop.sync_info = mybir.SyncInfo(on_wait=[w], on_update=[])
                        insts.insert(idx, nop)
                        idx += 1
                    inst.sync_info = mybir.SyncInfo(
                        on_wait=[waits[-1]], on_update=list(si.on_update or [])
                    )
                idx += 1


def build_nc(use_seq_codegen: bool = False):
    """Build the Bass program (identical on all 8 cores)."""
    key = ("nc", use_seq_codegen)
    if key in _CACHE:
        return _CACHE[key]
    import concourse.bass as bass
    import concourse.mybir as mybir
    from concourse.bass import _add_dep_helper
    from concourse.tile import TileContext

    _maybe_patch_ldw_opt()

    bf16 = mybir.dt.bfloat16
    f32 = mybir.dt.float32

    nc = bass.Bass(use_seq_codegen=use_seq_codegen)

    blob1_d = nc.declare_dram_parameter("blob1", [F + 1, _BLOB1_W], bf16, isOutput=False)
    blob2_d = nc.declare_dram_parameter("blob2", [N, _BLOB2_W], bf16, isOutput=False)
    # dT8[3*il+c, bank*128+j] = d[bank*8+il, j, c] — per-bank stationaries
    dT8_d = nc.declare_dram_parameter("dT8", [3 * NB, (N // NB) * N], bf16, isOutput=False)
    # zero-fill image for the block-diagonal p moving operand
    pz_d = nc.declare_dram_parameter("pz", [3 * NB, (N // NB) * NB * K], bf16, isOutput=False)
    out_d = nc.declare_dram_parameter("out", [N, N * K], bf16, isOutput=True)

    p_scratch = nc.dram_tensor("p_scratch", [N, C, K], bf16)

    NG = N // SC  # super-chunks
    NBK = SC // NB  # banks per super-chunk
    NBANKS = N // NB  # total banks

    with TileContext(nc) as tc:
        with (
            tc.tile_pool(name="const", bufs=1) as constp,
            tc.tile_pool(name="tprod", bufs=4) as tpool,
            tc.tile_pool(name="outsb", bufs=2) as outp,
            tc.tile_pool(name="psum", bufs=4, space="PSUM") as psump,
        ):
            # ---- input loads (projection inputs first on their own queue) ----
            blob1_sb = constp.tile([F + 1, _BLOB1_W], bf16)
            nc.scalar.dma_start(out=blob1_sb[:], in_=blob1_d[:])
            blob2_sb = constp.tile([N, _BLOB2_W], bf16)
            nc.sync.dma_start(out=blob2_sb[:], in_=blob2_d[:])
            dT8_sb = constp.tile([3 * NB, NBANKS * N], bf16)
            nc.sync.dma_start(out=dT8_sb[:], in_=dT8_d[:])
            # p_blk starts as zeros; the slot-DMAs below fill the diagonal
            p_blk = constp.tile([3 * NB, NBANKS, NB, K], bf16)
            nc.scalar.dma_start(out=p_blk[:], in_=pz_d[:])

            id_sb = blob2_sb[:, _ID_OFF : _ID_OFF + N]
            wp_sb = blob1_sb[0:F, _WP_OFF : _WP_OFF + K]
            wq_sb = blob1_sb[0 : F + 1, _WQ_OFF : _WQ_OFF + K]

            # ---- projections (q first: it gates the DVE product stream) ----
            p_ps = psump.tile([N, C * K], f32, tag="ps")
            q_ps = psump.tile([N, C * K], f32, tag="ps")
            for c in range(C):
                nc.tensor.matmul(
                    q_ps[:, c * K : (c + 1) * K],
                    lhsT=blob1_sb[0 : F + 1, _VFT_OFF + c * N : _VFT_OFF + (c + 1) * N],
                    rhs=wq_sb,
                    start=True,
                    stop=True,
                )
            for c in range(C):
                nc.tensor.matmul(
                    p_ps[:, c * K : (c + 1) * K],
                    lhsT=blob1_sb[0:F, _VFT_OFF + c * N : _VFT_OFF + (c + 1) * N],
                    rhs=wp_sb,
                    start=True,
                    stop=True,
                )
            # q drained by DVE so the DVE products inherit the PE sync by
            # program order; p drained by ACT, bounced through DRAM into the
            # 8 diagonal slots of p_blk.
            q_sb = constp.tile([N, C, K], bf16)
            nc.vector.tensor_copy(q_sb[:], q_ps[:])
            p_sb = constp.tile([N, C, K], bf16)
            nc.scalar.copy(p_sb[:], p_ps[:])
            nc.sync.dma_start(out=p_scratch[:], in_=p_sb[:])
            for il in range(NB):
                # p_blk[3*il+c, bank, il, k] = p[bank*8+il, c, k]
                # split across both HWDGE queues to halve issue serialization
                eng = nc.sync if il % 2 == 0 else nc.scalar
                eng.dma_start(
                    out=p_blk[3 * il : 3 * il + C, :, il, :],
                    in_=p_scratch[il::NB].transpose([1, 0, 2]),
                )

            # ---- software-pipelined main loop ----
            # per super-chunk: DVE products -> PE identity-sums (bank opener,
            # keeps PE fed/warm as soon as each product lands). The term-A
            # accumulate + relu-drain + out-DMA for chunk g are DEFERRED two
            # chunks so the PE stream never blocks on the late p_blk DMAs.
            def emit_products_sums(g):
                i0 = g * SC
                t_tiles = []
                for c in range(C):
                    tt = tpool.tile([N, SC, K], bf16, tag=f"t{c}")
                    # in0: d duplicated x2 along k-pairs -> innermost step-1
                    in0 = (
                        blob2_sb[
                            :,
                            _DD_OFF + (c * N + i0) * 2 : _DD_OFF + (c * N + i0 + SC) * 2,
                        ]
                        .rearrange("p (i k2) -> p i k2", k2=2)
                        .unsqueeze(2)
                        .broadcast_to([N, SC, K // 2, 2])
                    )
                    in1 = (
                        q_sb[:, c, :]
                        .rearrange("p (kh k2) -> p kh k2", k2=2)
                        .unsqueeze(1)
                        .broadcast_to([N, SC, K // 2, 2])
                    )
                    # all products on DVE: concurrent GPSIMD elementwise work
                    # contends for SBUF ports and slows BOTH engines below
                    # DVE's solo 2x rate
                    nc.vector.tensor_tensor(
                        out=tt[:].rearrange("p i (kh k2) -> p i kh k2", k2=2),
                        in0=in0,
                        in1=in1,
                        op=mybir.AluOpType.mult,
                    )
                    t_tiles.append(tt)

                ps = psump.tile([N, SC * K], f32, tag="ps")
                openers = []
                for h in range(2):  # the two 2KB banks of the double tile
                    opener = None
                    for c in range(C):
                        mm = nc.tensor.matmul(
                            ps[:, h * NB * K : (h + 1) * NB * K],
                            lhsT=id_sb,
                            rhs=t_tiles[c][:, h * NB : (h + 1) * NB, :],
                            start=(c == 0),
                            stop=False,
                            skip_group_check=True,
                        )
                        if opener is None:
                            opener = mm
                        else:
                            _add_dep_helper(mm.ins, opener.ins, False, "after-opener")
                    openers.append(opener)
                return ps, openers

            def emit_ta_drain(g, ps, openers):
                i0 = g * SC
                ob = outp.tile([N, SC * K], bf16, tag="ob")
                for h in range(2):
                    bank = g * 2 + h
                    ta = nc.tensor.matmul(
                        ps[:, h * NB * K : (h + 1) * NB * K],
                        lhsT=dT8_sb[:, bank * N : (bank + 1) * N],
                        rhs=p_blk[:, bank, :, :],
                        start=False,
                        stop=True,
                        skip_group_check=True,
                    )
                    _add_dep_helper(ta.ins, openers[h].ins, False, "ta-after-opener")
                if g >= NG - 2:
                    # the DVE is idle by the time the last chunks drain; relu
                    # there so the ACT drain chain isn't the tail
                    nc.vector.tensor_relu(out=ob[:], in_=ps[:])
                else:
                    nc.scalar.activation(
                        ob[:], ps[:], func=mybir.ActivationFunctionType.Relu
                    )
                nc.sync.dma_start(out=out_d[:, i0 * K : (i0 + SC) * K], in_=ob[:])

            DEFER = 1
            pending = []
            for g in range(NG):
                pending.append((g, *emit_products_sums(g)))
                if len(pending) > DEFER:
                    emit_ta_drain(*pending.pop(0))
            for item in pending:
                emit_ta_drain(*item)

    _legalize_waits(nc)
    _CACHE[key] = nc
    return nc


def prep_core_inputs(vf_b: np.ndarray, d_b: np.ndarray, w: np.ndarray, b: np.ndarray):
    """Host-side shard prep for one core (layout transforms only)."""
    import ml_dtypes

    bf16 = ml_dtypes.bfloat16
    blob1 = np.zeros((F + 1, _BLOB1_W), dtype=np.float32)
    # vfT[f, c*128+n] = vf[n, c, f]; row F = ones (bias row)
    blob1[0:F, _VFT_OFF : _VFT_OFF + C * N] = vf_b.transpose(2, 1, 0).reshape(F, C * N)
    blob1[F, _VFT_OFF : _VFT_OFF + C * N] = 1.0
    blob1[0:F, _WP_OFF : _WP_OFF + K] = w[:F]
    blob1[0:F, _WQ_OFF : _WQ_OFF + K] = w[F:]
    blob1[F, _WQ_OFF : _WQ_OFF + K] = b
    blob2 = np.zeros((N, _BLOB2_W), dtype=np.float32)
    # d_dup2[j, ((c,i),k2)] = d[i,j,c] for k2 in {0,1}
    dd = d_b.transpose(1, 2, 0).reshape(N, C * N)  # [j, (c,i)]
    blob2[:, _DD_OFF : _DD_OFF + 2 * C * N] = np.repeat(dd, 2, axis=1)
    blob2[:, _ID_OFF : _ID_OFF + N] = np.eye(N, dtype=np.float32)
    # dT8[3*il+c, bank*128+j] = d[bank*8+il, j, c]
    dT8 = np.empty((3 * NB, N // NB, N), dtype=np.float32)
    dT = d_b.transpose(2, 0, 1)  # [c, i, j]
    for il in range(NB):
        for c in range(C):
            dT8[3 * il + c] = dT[c, il::NB, :]
    dT8 = dT8.reshape(3 * NB, (N // NB) * N).astype(bf16)
    pz = np.zeros((3 * NB, (N // NB) * NB * K), dtype=bf16)
    return {
        "blob1": blob1.astype(bf16),
        "blob2": blob2.astype(bf16),
        "dT8": dT8,
        "pz": pz,
    }


def prep_all_inputs(inputs: dict):
    vf = np.asarray(inputs["vector_features"], dtype=np.float32)
    d = np.asarray(inputs["distances"], dtype=np.float32)
    w = np.asarray(inputs["w_vs"], dtype=np.float32)
    b = np.asarray(inputs["b_vs"], dtype=np.float32)
    return [prep_core_inputs(vf[i], d[i], w, b) for i in range(B)]


def gather_output(results: list) -> np.ndarray:
    """results[b]['out'] is [j, (i,k)] bf16 -> full [B,N,N,K] f32."""
    out = np.empty((B, N, N, K), dtype=np.float32)
    for bidx in range(B):
        o = np.asarray(results[bidx]["out"]).astype(np.float32)
        out[bidx] = o.reshape(N, N, K).transpose(1, 0, 2)
    return out


def _numpy_reference(vf, d, w, b):
    w_i, w_j = w[:F], w[F:]
    p = np.einsum("bncf,fk->bnck", vf, w_i)
    q = np.einsum("bncf,fk->bnck", vf, w_j) + b
    s = np.einsum("bick,bijc->bijk", p, d) + np.einsum("bjck,bijc->bijk", q, d)
    return np.maximum(s, 0.0).astype(np.float32)


def kernel(**inputs: np.ndarray) -> np.ndarray:
    vf = np.asarray(inputs["vector_features"], dtype=np.float32)
    d = np.asarray(inputs["distances"], dtype=np.float32)
    w = np.asarray(inputs["w_vs"], dtype=np.float32)
    b = np.asarray(inputs["b_vs"], dtype=np.float32)

    if not _import_bass():
        return _numpy_reference(vf, d, w, b)

    try:
        from concourse.bass_utils import run_bass_kernel_spmd

        nc = build_nc()
        in_maps = prep_all_inputs(inputs)
        res = run_bass_kernel_spmd(nc, in_maps, core_ids=list(range(_N_CORES)))
        return gather_output(res.results)
    except Exception as e:  # defensive: keep grading alive if HW path breaks
        import traceback

        traceback.print_exc()
        print(f"WARNING: bass path failed ({e}); falling back to numpy")
        return _numpy_reference(vf, d, w, b)


if __name__ == "__main__":
    rng = np.random.default_rng(0)
    ins = {
        "vector_features": rng.standard_normal((B, N, C, F)).astype(np.float32),
        "distances": rng.standard_normal((B, N, N, C)).astype(np.float32),
        "w_vs": (rng.standard_normal((2 * F, K)) / np.sqrt(2 * F)).astype(np.float32),
        "b_vs": np.zeros((K,), dtype=np.float32),
    }
    out = kernel(**ins)
    exp = _numpy_reference(
        ins["vector_features"], ins["distances"], ins["w_vs"], ins["b_vs"]
    )
    rel = np.abs(out - exp).max() / (np.abs(exp).max() + 1e-12)
    print("shape", out.shape, "rel", rel)


# revision 61
# speedup vs baseline: 1.1359x; 1.1359x over previous
"""Trainium2 Bass kernel for nn_GraphVToS_9388798509586 (gnn_message_passing).

Math (per batch element b):
    out[i,j,k] = relu( sum_c d[i,j,c] * (p[i,c,k] + q[j,c,k]) )
    p = vf @ w_vs[:F]
    q = vf @ w_vs[F:] + b_vs     (bias folded: sum_c d[i,j,c]*b[k])

Sharding: data-parallel over batch B=8, one element per NeuronCore.

Per-core device schedule (dense [j, (i,k)] accumulation layout):
  - PE computes projections p, q (6 small matmuls, bias via ones-row).
  - Pairwise term B (sum_c d[i,j,c]*q[j,c,k], elementwise in j) is computed
    as three broadcast products t_c[j,(i,k)] = d[i,j,c]*q[j,c,k] on
    DVE/GPSIMD. d is host-duplicated x2 along k-pairs so BOTH operands read
    innermost step-1 bf16, which unlocks the DVE 2x perf mode. The c-sum
    happens FOR FREE in PSUM by streaming each t_c through the PE with an
    identity stationary (out += I.T @ t_c).
  - Term A (sum_c d[i,j,c]*p[i,c,k]) is ONE matmul per 8-i PSUM bank:
    stationary dT8 [24,128] (rows 3*il+c = d[i0+il,:,c]) against a
    block-diagonal moving operand p_blk [24, (il,k)] holding p[i0+il,c,k]
    in slot il and zeros elsewhere. N=512 streams hide all LDWEIGHTS.
  - ACT drains PSUM with fused ReLU to bf16; 4 batched DMAs out.
  - Software pipeline: term-A + products for chunk g overlap; identity-sums
    for g-1 follow.
Output leaves the device as out[j, i, k] bf16; the host transposes to
[i,j,k] and casts to f32 (layout/gather only, no math).

kernel() is self-contained: hardcoded shapes, host-side shard prep + gather.
"""

import os

import numpy as np

B, N, C, F, K = 8, 128, 3, 64, 64
_N_CORES = 8

NB = 8  # i's per PSUM bank (8*64 = 512 cols)
SC = 16  # i's per super-chunk (2 banks = 1 double-bank PSUM tile)

# blob1 (small, loads first; feeds projections): [vfT | wp | wq]
_VFT_OFF = 0  # rows 0:65: vfT[f, c*128+n]   cols 0:384
_WP_OFF = 384  # rows 0:64                     cols 384:448
_WQ_OFF = 448  # rows 0:65                     cols 448:512
_BLOB1_W = 512
# blob2: [d_dup2 | ident]
_DD_OFF = 0  # [j, (c,i,k2)] = d[i,j,c] duplicated x2   cols 0:768
_ID_OFF = 768  # identity 128x128                         cols 768:896
_BLOB2_W = 896

_BASS_READY = None
_CACHE = {}


def _import_bass():
    global _BASS_READY
    if _BASS_READY is None:
        try:
            import sys

            for p in ("/opt/trn_rl_repo",):
                if p not in sys.path:
                    sys.path.insert(0, p)
            import concourse.bass  # noqa: F401

            _BASS_READY = True
        except Exception:
            _BASS_READY = False
    return _BASS_READY


def _maybe_patch_ldw_opt():
    """Flip walrus --enable-ldw-opt. Kill switch: KERNEL_LDW_OPT=0."""
    import concourse.bass_utils as _bu

    if getattr(_bu, "_ldwopt_patched", False):
        return
    if os.environ.get("KERNEL_LDW_OPT", "1") != "1":
        return
    _orig = _bu.get_walrus_args

    def _gwa(*a, **k):
        return [
            x.replace("--enable-ldw-opt=false", "--enable-ldw-opt=true")
            for x in _orig(*a, **k)
        ]

    _bu.get_walrus_args = _gwa
    _bu._ldwopt_patched = True


def _legalize_waits(nc):
    """Split multi-semaphore waits onto same-engine NOP carriers.

    This walrus build encodes at most ONE sync-wait per compute instruction
    (setupSyncWait raises "Too many sync wait commands" otherwise), and the
    Tile scheduler happily emits 2-3. Inserting a NOP right before the
    instruction on the same engine is semantics-preserving: the engine would
    have blocked at that point anyway.
    """
    import concourse.mybir as mybir

    nop_ctr = [0]

    def make_nop(engine):
        bi = nc.engines[engine].nop(nofuse=True)
        inst = bi.ins
        for f in nc.m.functions:
            for blk in f.blocks:
                try:
                    blk.instructions.remove(inst)
                except ValueError:
                    pass
        inst.name = f"{inst.name}-legalize-{nop_ctr[0]}"
        nop_ctr[0] += 1
        return inst

    for f in nc.m.functions:
        for blk in f.blocks:
            insts = blk.instructions
            idx = 0
            while idx < len(insts):
                inst = insts[idx]
                si = inst.sync_info
                waits = list(si.on_wait) if si is not None and si.on_wait else []
                if len(waits) > 1:
                    for w in waits[:-1]:
                        nop = make_nop(inst.engine)
                        nop.sync_info = mybir.SyncInfo(on_wait=[w], on_update=[])
                        insts.insert(idx, nop)
                        idx += 1
                    inst.sync_info = mybir.SyncInfo(
                        on_wait=[waits[-1]], on_update=list(si.on_update or [])
                    )
                idx += 1


def build_nc(use_seq_codegen: bool = False):
    """Build the Bass program (identical on all 8 cores)."""
    key = ("nc", use_seq_codegen)
    if key in _CACHE:
        return _CACHE[key]
    import concourse.bass as bass
    import concourse.mybir as mybir
    from concourse.bass import _add_dep_helper
    from concourse.tile import TileContext

    _maybe_patch_ldw_opt()

    bf16 = mybir.dt.bfloat16
    f32 = mybir.dt.float32

    nc = bass.Bass(use_seq_codegen=use_seq_codegen)

    blob1_d = nc.declare_dram_parameter("blob1", [F + 1, _BLOB1_W], bf16, isOutput=False)
    blob2_d = nc.declare_dram_parameter("blob2", [N, _BLOB2_W], bf16, isOutput=False)
    # dT8[3*il+c, bank*128+j] = d[bank*8+il, j, c] — per-bank stationaries
    dT8_d = nc.declare_dram_parameter("dT8", [3 * NB, (N // NB) * N], bf16, isOutput=False)
    # zero-fill image for the block-diagonal p moving operand
    pz_d = nc.declare_dram_parameter("pz", [3 * NB, (N // NB) * NB * K], bf16, isOutput=False)
    out_d = nc.declare_dram_parameter("out", [N, N * K], bf16, isOutput=True)

    p_scratch = nc.dram_tensor("p_scratch", [N, C, K], bf16)

    NG = N // SC  # super-chunks
    NBK = SC // NB  # banks per super-chunk
    NBANKS = N // NB  # total banks

    with TileContext(nc) as tc:
        with (
            tc.tile_pool(name="const", bufs=1) as constp,
            tc.tile_pool(name="tprod", bufs=4) as tpool,
            tc.tile_pool(name="outsb", bufs=2) as outp,
            tc.tile_pool(name="psum", bufs=4, space="PSUM") as psump,
        ):
            # ---- input loads (projection inputs first on their own queue) ----
            blob1_sb = constp.tile([F + 1, _BLOB1_W], bf16)
            nc.scalar.dma_start(out=blob1_sb[:], in_=blob1_d[:])
            blob2_sb = constp.tile([N, _BLOB2_W], bf16)
            nc.sync.dma_start(out=blob2_sb[:], in_=blob2_d[:])
            dT8_sb = constp.tile([3 * NB, NBANKS * N], bf16)
            nc.sync.dma_start(out=dT8_sb[:], in_=dT8_d[:])
            # p_blk starts as zeros; the slot-DMAs below fill the diagonal
            p_blk = constp.tile([3 * NB, NBANKS, NB, K], bf16)
            nc.scalar.dma_start(out=p_blk[:], in_=pz_d[:])

            id_sb = blob2_sb[:, _ID_OFF : _ID_OFF + N]
            wp_sb = blob1_sb[0:F, _WP_OFF : _WP_OFF + K]
            wq_sb = blob1_sb[0 : F + 1, _WQ_OFF : _WQ_OFF + K]

            # ---- projections (q first: it gates the DVE product stream) ----
            p_ps = psump.tile([N, C * K], f32, tag="ps")
            q_ps = psump.tile([N, C * K], f32, tag="ps")
            for c in range(C):
                nc.tensor.matmul(
                    q_ps[:, c * K : (c + 1) * K],
                    lhsT=blob1_sb[0 : F + 1, _VFT_OFF + c * N : _VFT_OFF + (c + 1) * N],
                    rhs=wq_sb,
                    start=True,
                    stop=True,
                )
            for c in range(C):
                nc.tensor.matmul(
                    p_ps[:, c * K : (c + 1) * K],
                    lhsT=blob1_sb[0:F, _VFT_OFF + c * N : _VFT_OFF + (c + 1) * N],
                    rhs=wp_sb,
                    start=True,
                    stop=True,
                )
            # q drained by DVE so the DVE products inherit the PE sync by
            # program order; p drained by ACT, bounced through DRAM into the
            # 8 diagonal slots of p_blk.
            q_sb = constp.tile([N, C, K], bf16)
            nc.vector.tensor_copy(q_sb[:], q_ps[:])
            p_sb = constp.tile([N, C, K], bf16)
            nc.scalar.copy(p_sb[:], p_ps[:])
            nc.sync.dma_start(out=p_scratch[:], in_=p_sb[:])
            for il in range(NB):
                # p_blk[3*il+c, bank, il, k] = p[bank*8+il, c, k]
                # split across both HWDGE queues to halve issue serialization
                eng = nc.sync if il % 2 == 0 else nc.scalar
                eng.dma_start(
                    out=p_blk[3 * il : 3 * il + C, :, il, :],
                    in_=p_scratch[il::NB].transpose([1, 0, 2]),
                )

            # ---- software-pipelined main loop ----
            # per super-chunk: DVE products -> PE identity-sums (bank opener,
            # keeps PE fed/warm as soon as each product lands). The term-A
            # accumulate + relu-drain + out-DMA for chunk g are DEFERRED two
            # chunks so the PE stream never blocks on the late p_blk DMAs.
            def emit_products_sums(g):
                i0 = g * SC
                t_tiles = []
                for c in range(C):
                    tt = tpool.tile([N, SC, K], bf16, tag=f"t{c}")
                    # in0: d duplicated x2 along k-pairs -> innermost step-1
                    in0 = (
                        blob2_sb[
                            :,
                            _DD_OFF + (c * N + i0) * 2 : _DD_OFF + (c * N + i0 + SC) * 2,
                        ]
                        .rearrange("p (i k2) -> p i k2", k2=2)
                        .unsqueeze(2)
                        .broadcast_to([N, SC, K // 2, 2])
                    )
                    in1 = (
                        q_sb[:, c, :]
                        .rearrange("p (kh k2) -> p kh k2", k2=2)
                        .unsqueeze(1)
                        .broadcast_to([N, SC, K // 2, 2])
                    )
                    # all products on DVE: concurrent GPSIMD elementwise work
                    # contends for SBUF ports and slows BOTH engines below
                    # DVE's solo 2x rate
                    nc.vector.tensor_tensor(
                        out=tt[:].rearrange("p i (kh k2) -> p i kh k2", k2=2),
                        in0=in0,
                        in1=in1,
                        op=mybir.AluOpType.mult,
                    )
                    t_tiles.append(tt)

                ps = psump.tile([N, SC * K], f32, tag="ps")
                openers = []
                for h in range(2):  # the two 2KB banks of the double tile
                    opener = None
                    for c in range(C):
                        mm = nc.tensor.matmul(
                            ps[:, h * NB * K : (h + 1) * NB * K],
                            lhsT=id_sb,
                            rhs=t_tiles[c][:, h * NB : (h + 1) * NB, :],
                            start=(c == 0),
                            stop=False,
                            skip_group_check=True,
                        )
                        if opener is None:
                            opener = mm
                        else:
                            _add_dep_helper(mm.ins, opener.ins, False, "after-opener")
                    openers.append(opener)
                return ps, openers

            def emit_ta_drain(g, ps, openers):
                i0 = g * SC
                ob = outp.tile([N, SC * K], bf16, tag="ob")
                for h in range(2):
                    bank = g * 2 + h
                    ta = nc.tensor.matmul(
                        ps[:, h * NB * K : (h + 1) * NB * K],
                        lhsT=dT8_sb[:, bank * N : (bank + 1) * N],
                        rhs=p_blk[:, bank, :, :],
                        start=False,
                        stop=True,
                        skip_group_check=True,
                    )
                    _add_dep_helper(ta.ins, openers[h].ins, False, "ta-after-opener")
                if g >= NG - 2:
                    # the DVE is idle by the time the last chunks drain; relu
                    # there so the ACT drain chain isn't the tail
                    nc.vector.tensor_relu(out=ob[:], in_=ps[:])
                else:
                    nc.scalar.activation(
                        ob[:], ps[:], func=mybir.ActivationFunctionType.Relu
                    )
                nc.sync.dma_start(out=out_d[:, i0 * K : (i0 + SC) * K], in_=ob[:])

            DEFER = 2
            pending = []
            for g in range(NG):
                pending.append((g, *emit_products_sums(g)))
                if len(pending) > DEFER:
                    emit_ta_drain(*pending.pop(0))
            for item in pending:
                emit_ta_drain(*item)

    _legalize_waits(nc)
    _CACHE[key] = nc
    return nc


def prep_core_inputs(vf_b: np.ndarray, d_b: np.ndarray, w: np.ndarray, b: np.ndarray):
    """Host-side shard prep for one core (layout transforms only)."""
    import ml_dtypes

    bf16 = ml_dtypes.bfloat16
    blob1 = np.zeros((F + 1, _BLOB1_W), dtype=np.float32)
    # vfT[f, c*128+n] = vf[n, c, f]; row F = ones (bias row)
    blob1[0:F, _VFT_OFF : _VFT_OFF + C * N] = vf_b.transpose(2, 1, 0).reshape(F, C * N)
    blob1[F, _VFT_OFF : _VFT_OFF + C * N] = 1.0
    blob1[0:F, _WP_OFF : _WP_OFF + K] = w[:F]
    blob1[0:F, _WQ_OFF : _WQ_OFF + K] = w[F:]
    blob1[F, _WQ_OFF : _WQ_OFF + K] = b
    blob2 = np.zeros((N, _BLOB2_W), dtype=np.float32)
    # d_dup2[j, ((c,i),k2)] = d[i,j,c] for k2 in {0,1}
    dd = d_b.transpose(1, 2, 0).reshape(N, C * N)  # [j, (c,i)]
    blob2[:, _DD_OFF : _DD_OFF + 2 * C * N] = np.repeat(dd, 2, axis=1)
    blob2[:, _ID_OFF : _ID_OFF + N] = np.eye(N, dtype=np.float32)
    # dT8[3*il+c, bank*128+j] = d[bank*8+il, j, c]
    dT8 = np.empty((3 * NB, N // NB, N), dtype=np.float32)
    dT = d_b.transpose(2, 0, 1)  # [c, i, j]
    for il in range(NB):
        for c in range(C):
            dT8[3 * il + c] = dT[c, il::NB, :]
    dT8 = dT8.reshape(3 * NB, (N // NB) * N).astype(bf16)
    pz = np.zeros((3 * NB, (N // NB) * NB * K), dtype=bf16)
    return {
        "blob1": blob1.astype(bf16),
        "blob2": blob2.astype(bf16),
        "dT8": dT8,
        "pz": pz,
    }


def prep_all_inputs(inputs: dict):
    vf = np.asarray(inputs["vector_features"], dtype=np.float32)
    d = np.asarray(inputs["distances"], dtype=np.float32)
    w = np.asarray(inputs["w_vs"], dtype=np.float32)
    b = np.asarray(inputs["b_vs"], dtype=np.float32)
    return [prep_core_inputs(vf[i], d[i], w, b) for i in range(B)]


def gather_output(results: list) -> np.ndarray:
    """results[b]['out'] is [j, (i,k)] bf16 -> full [B,N,N,K] f32."""
    out = np.empty((B, N, N, K), dtype=np.float32)
    for bidx in range(B):
        o = np.asarray(results[bidx]["out"]).astype(np.float32)
        out[bidx] = o.reshape(N, N, K).transpose(1, 0, 2)
    return out


def _numpy_reference(vf, d, w, b):
    w_i, w_j = w[:F], w[F:]
    p = np.einsum("bncf,fk->bnck", vf, w_i)
    q = np.einsum("bncf,fk->bnck", vf, w_j) + b
    s = np.einsum("bick,bijc->bijk", p, d) + np.einsum("bjck,bijc->bijk", q, d)
    return np.maximum(s, 0.0).astype(np.float32)


def kernel(**inputs: np.ndarray) -> np.ndarray:
    vf = np.asarray(inputs["vector_features"], dtype=np.float32)
    d = np.asarray(inputs["distances"], dtype=np.float32)
    w = np.asarray(inputs["w_vs"], dtype=np.float32)
    b = np.asarray(inputs["b_vs"], dtype=np.float32)

    if not _import_bass():
        return _numpy_reference(vf, d, w, b)

    try:
        from concourse.bass_utils import run_bass_kernel_spmd

        nc = build_nc()
        in_maps = prep_all_inputs(inputs)
        res = run_bass_kernel_spmd(nc, in_maps, core_ids=list(range(_N_CORES)))
        return gather_output(res.results)
    except Exception as e:  # defensive: keep grading alive if HW path breaks
        import traceback

        traceback.print_exc()
        print(f"WARNING: bass path failed ({e}); falling back to numpy")
        return _numpy_reference(vf, d, w, b)


if __name__ == "__main__":
    rng = np.random.default_rng(0)
    ins = {
        "vector_features": rng.standard_normal((B, N, C, F)).astype(np.float32),
        "distances": rng.standard_normal((B, N, N, C)).astype(np.float32),
        "w_vs": (rng.standard_normal((2 * F, K)) / np.sqrt(2 * F)).astype(np.float32),
        "b_vs": np.zeros((K,), dtype=np.float32),
    }
    out = kernel(**ins)
    exp = _numpy_reference(
        ins["vector_features"], ins["distances"], ins["w_vs"], ins["b_vs"]
    )
    rel = np.abs(out - exp).max() / (np.abs(exp).max() + 1e-12)
    print("shape", out.shape, "rel", rel)
